# revision 1
# baseline (speedup 1.0000x reference)
"""Trainium2 Bass kernel for nn_DenseAttnProcessor (sparse_attention).

Cross-attention block: q = hs@Wq, k/v = ehs@{Wk,Wv}, per-head softmax((q k^T)/8
+ col_bias) @ v, @Wo + bo + residual.  B=8 batches -> data-parallel, one batch
per NeuronCore (no collectives).

Per-core dataflow (everything lives in "transposed" orientation so that every
matmul contraction has its operand already partition-major; softmax runs on
scoresT [T, q] with the per-head denominator handled by a ones-column matmul, a
reciprocal, and a K=1 broadcast matmul):

  stage A (once):  ehsT (host-pretransposed, bf16) -> k,v [77,1024] via matmul
                   -> kT via PE transpose -> M_h = v_h @ Wo_h [77,1024];
                   M rows DMA-packed into a [16*77+1, 1024] stack; the +bo
                   term rides as an extra stack row paired with an all-ones
                   probs row.
  stage B (8 chunks of 512 q rows):
                   hs chunk f32 -> bf16 cast -> XBAR DMA-transpose -> hsT [C, q]
                   qT = Wq^T@hsT (psum accum over C) [inner, q]
                   per head: scoresT [77,512] = kT_h^T qT_h; z = Exp(scoresT)
                   * exp(col_bias)^T (host-precomputed multiplicative mask,
                   exact "set-column" suppression semantics, rows without
                   suppression are exactly 1.0); D = ones^T z; Dinv via fast
                   DVE reciprocal; DinvB via K=1 broadcast matmul; probsT =
                   z * DinvB, DMA-packed into the [16*77+1, 512] stack;
                   out[q,C] = sum_kt probsT_kt^T @ M_kt (10 K=128 matmuls)
                   + residual (f32) -> DMA out.  Chunks are software-
                   pipelined: softmax(ci-1) is emitted interleaved with
                   qT(ci) so the PE stream stays dense (HAM stays warm).

Inputs are the full unsharded arrays as produced by setup_inputs(); host side
only shards/casts/transposes small tensors and computes the tiny [2,77]/[2,4096]
suppression vectors.
"""

import sys

for _p in ("/opt/trn_rl_repo",):
    if _p not in sys.path:
        sys.path.insert(0, _p)

import numpy as np
import ml_dtypes

import concourse.mybir as mybir
import concourse.tile as tile
from concourse import bacc
from concourse.bass import ds
from concourse.masks import make_identity

F32 = mybir.dt.float32
BF16 = mybir.dt.bfloat16
AF = mybir.ActivationFunctionType

B, HW, C, CT, T, H, D = 8, 4096, 1024, 2048, 77, 16, 64
SUPPRESS = 20.0
RT = H * T + 1                # 1233 stacked rows (16*77 head rows + bo row)
NKT = (RT + 127) // 128       # 10 K-tiles for the AV matmul
NQ = 512                      # q rows per chunk
NCHUNK = HW // NQ             # 8
BO_TILE, BO_PART = (H * T) // 128, (H * T) % 128   # bo/ones row: tile 9, p 80


def _pack_pieces(h):
    """DMA pieces for packing head h's 77 rows at stacked row 77*h, split at
    128-row tile boundaries.  Returns list of (tile_idx, part_base, src_start,
    nrows).  (DMA writes have no partition-alignment restrictions.)"""
    g = T * h
    pieces = []
    pos = 0
    while pos < T:
        gg = g + pos
        ti, d = gg // 128, gg % 128
        n = min(T - pos, 128 - d)
        pieces.append((ti, d, pos, n))
        pos += n
    return pieces


def build_nc():
    nc = bacc.Bacc("TRN2", target_bir_lowering=False, debug=False)

    hs = nc.dram_tensor("hs", [HW, C], F32, kind="ExternalInput")
    ehsT = nc.dram_tensor("ehsT", [CT, T], BF16, kind="ExternalInput")
    wq = nc.dram_tensor("wq", [C, C], BF16, kind="ExternalInput")
    wk = nc.dram_tensor("wk", [CT, C], BF16, kind="ExternalInput")
    wv = nc.dram_tensor("wv", [CT, C], BF16, kind="ExternalInput")
    wo = nc.dram_tensor("wo", [C, C], BF16, kind="ExternalInput")
    euabt = nc.dram_tensor("euabt", [T, HW], BF16, kind="ExternalInput")
    bo = nc.dram_tensor("bo", [1, C], BF16, kind="ExternalInput")
    out = nc.dram_tensor("out", [HW, C], F32, kind="ExternalOutput")

    with tile.TileContext(nc) as tc:
        with (
            tc.tile_pool(name="const", bufs=1) as const,
            tc.tile_pool(name="persist", bufs=1) as persist,
        ):
            ident = const.tile([128, 128], BF16)
            make_identity(nc, ident)
            ones_col = const.tile([T, 1], BF16)
            nc.any.memset(ones_col, 1.0)
            ones_row = const.tile([1, T], BF16)
            nc.any.memset(ones_row, 1.0)
            ones_q = const.tile([1, NQ], BF16)
            nc.any.memset(ones_q, 1.0)
            eu_sb = const.tile([T, HW], BF16)
            nc.sync.dma_start(eu_sb, euabt[:, :])

            # persistent stacks
            kT_sb = persist.tile([128, C // 128, T], BF16)        # [inner, t]
            m_tiles = [persist.tile([128, C], BF16, name=f"m{i}") for i in range(NKT)]
            prob_bufs = [
                [persist.tile([128, NQ], BF16, name=f"pb{b}_{i}") for i in range(NKT)]
                for b in range(2)
            ]
            wq_tiles = [persist.tile([128, C], BF16, name=f"wqt{i}") for i in range(C // 128)]
            for i in range(C // 128):
                nc.sync.dma_start(wq_tiles[i], wq[ds(128 * i, 128), :])

            # only the last stack tile has rows past the packed head rows;
            # zero it so the AV matmuls see zeros there, then land bo/ones.
            nc.any.memset(m_tiles[BO_TILE], 0.0)
            for bset in prob_bufs:
                nc.any.memset(bset[BO_TILE], 0.0)
                nc.sync.dma_start(
                    bset[BO_TILE][BO_PART : BO_PART + 1, :], ones_q
                )
            nc.sync.dma_start(m_tiles[BO_TILE][BO_PART : BO_PART + 1, :], bo[:, :])

            # ---------------- stage A: k, v, kT, M ----------------
            with (
                tc.tile_pool(name="sa_sb", bufs=3) as sa_sb,
                tc.tile_pool(name="sa_w", bufs=3) as sa_w,
                tc.tile_pool(name="sa_ps", bufs=2, space="PSUM") as sa_ps,
            ):
                ehsT_sb = sa_sb.tile([128, CT // 128, T], BF16, bufs=1)
                for j in range(CT // 128):
                    nc.sync.dma_start(ehsT_sb[:, j, :], ehsT[ds(128 * j, 128), :])

                kv_sb = {}
                for name, wten in (("k", wk), ("v", wv)):
                    kv_ps = sa_ps.tile([T, C], F32, tag="kvps", bufs=1)
                    for j in range(CT // 128):
                        wt = sa_w.tile([128, C], BF16, tag="wkv")
                        nc.sync.dma_start(wt, wten[ds(128 * j, 128), :])
                        for nh in range(2):
                            nc.tensor.matmul(
                                kv_ps[:, ds(512 * nh, 512)],
                                ehsT_sb[:, j, :],
                                wt[:, ds(512 * nh, 512)],
                                start=(j == 0),
                                stop=(j == CT // 128 - 1),
                            )
                    kvs = sa_sb.tile([T, C], BF16, tag=f"{name}sb", bufs=1)
                    nc.any.tensor_copy(kvs, kv_ps)
                    kv_sb[name] = kvs

                # kT / vT via PE transpose of 128-column slices
                vT_sb = sa_sb.tile([128, C // 128, T], BF16, bufs=1)
                for src, dst in ((kv_sb["k"], kT_sb), (kv_sb["v"], vT_sb)):
                    for i in range(C // 128):
                        tp = sa_ps.tile([128, T], BF16, tag="tpa")
                        nc.tensor.transpose(tp, src[:, ds(128 * i, 128)], ident[:T, :T])
                        nc.any.tensor_copy(dst[:, i, :], tp)

                # M_h = v_h @ Wo_h, packed at stacked row 96h (+ bo at row 95)
                wot = None
                for h in range(H):
                    i, po = h // 2, (h % 2) * 64
                    if h % 2 == 0:
                        wot = sa_w.tile([128, C], BF16, tag="wot")
                        nc.sync.dma_start(wot, wo[ds(128 * i, 128), :])
                    m_ps = sa_ps.tile([T, C], F32, tag="mps")
                    for nh in range(2):
                        nc.tensor.matmul(
                            m_ps[:, ds(512 * nh, 512)],
                            vT_sb[ds(po, 64), i, :],
                            wot[ds(po, 64), ds(512 * nh, 512)],
                            start=True,
                            stop=True,
                        )
                    m_stg = sa_sb.tile([T, C], BF16, tag="mstg")
                    nc.any.tensor_copy(m_stg, m_ps)
                    for (ti, pb, s0, nr) in _pack_pieces(h):
                        nc.gpsimd.dma_start(
                            m_tiles[ti][ds(pb, nr), :], m_stg[ds(s0, nr), :]
                        )

            # ---------------- stage B: software-pipelined q chunks ----------------
            # Engine streams execute in emission order, so softmax(ci-1) head
            # chains are interleaved with qT(ci) matmul groups at build time:
            # the PE stream then always has dense matmul work queued and the
            # HAM clock gate stays open.
            with (
                tc.tile_pool(name="hsp", bufs=2) as hsp,
                tc.tile_pool(name="work", bufs=2) as work,
                tc.tile_pool(name="soft", bufs=4) as soft,
                tc.tile_pool(name="ops", bufs=2, space="PSUM") as ops,
            ):
                st = {}

                def load(ci):
                    q0 = NQ * ci
                    hs_f = hsp.tile([128, NQ // 128, C], F32, tag="hsf")
                    for qj in range(NQ // 128):
                        nc.sync.dma_start(
                            hs_f[:, qj, :], hs[ds(q0 + 128 * qj, 128), :]
                        )
                    hs_bf = work.tile([128, NQ // 128, C], BF16, tag="hsbf")
                    for qj in range(NQ // 128):
                        nc.scalar.copy(hs_bf[:, qj, :], hs_f[:, qj, :])
                    hsT = work.tile([128, C // 128, NQ], BF16, tag="hsT")
                    for qj in range(NQ // 128):
                        nc.sync.dma_start(
                            hsT[:, :, ds(128 * qj, 128)],
                            hs_bf[:, qj, :],
                            transpose=True,
                        )
                    qT = work.tile([128, C // 128, NQ], BF16, tag="qT")
                    st[ci] = dict(hs_f=hs_f, hsT=hsT, qT=qT)

                def qt_group(ci, ij):
                    hsT, qT = st[ci]["hsT"], st[ci]["qT"]
                    q_ps = ops.tile([128, NQ], F32, tag="qps", bufs=1)
                    for cj in range(C // 128):
                        nc.tensor.matmul(
                            q_ps,
                            wq_tiles[cj][:, ds(128 * ij, 128)],
                            hsT[:, cj, :],
                            start=(cj == 0),
                            stop=(cj == C // 128 - 1),
                        )
                    nc.any.tensor_copy(qT[:, ij, :], q_ps)

                def sm_head1(ci, h):
                    q0 = NQ * ci
                    qT = st[ci]["qT"]
                    i, po = h // 2, (h % 2) * 64
                    sT_ps = ops.tile([T, NQ], F32, tag="sT", bufs=2)
                    nc.tensor.matmul(
                        sT_ps,
                        kT_sb[ds(po, 64), i, :],
                        qT[ds(po, 64), i, :],
                        start=True,
                        stop=True,
                    )
                    expT = soft.tile([T, NQ], BF16, tag="expT", bufs=4)
                    nc.scalar.activation(expT, sT_ps, AF.Exp)
                    # multiplicative suppression mask exp(col_bias^T), host-
                    # precomputed; rows without suppression are exactly 1.0
                    z = soft.tile([T, NQ], BF16, tag="z", bufs=16, name=f"z{h}")
                    nc.vector.tensor_mul(z, expT, eu_sb[:, ds(q0, NQ)])
                    st[ci].setdefault("z", {})[h] = z

                def emit_d(ci, h):
                    d_ps = ops.tile([1, NQ], F32, tag="dps", bufs=2, name=f"dps{h}")
                    nc.tensor.matmul(d_ps, ones_col, st[ci]["z"][h], start=True, stop=True)
                    return d_ps

                def sm_head2(ci, h, d_ps, d_next):
                    prob = prob_bufs[ci % 2]
                    z = st[ci]["z"][h]
                    dinv = soft.tile([1, NQ], F32, tag="dinv", bufs=2)
                    nc.vector.reciprocal_approx_fast(dinv, d_ps)
                    dinv_bf = soft.tile([1, NQ], BF16, tag="dinvbf", bufs=2)
                    nc.scalar.copy(dinv_bf, dinv)
                    nxt = emit_d(ci, h + 1) if d_next else None
                    db_ps = ops.tile([T, NQ], F32, tag="db", bufs=1)
                    nc.tensor.matmul(db_ps, ones_row, dinv_bf, start=True, stop=True)
                    p_stg = soft.tile([T, NQ], BF16, tag="pstg", bufs=4)
                    nc.vector.tensor_mul(p_stg, z, db_ps)
                    for (ti, pb, s0, nr) in _pack_pieces(h):
                        nc.sync.dma_start(
                            prob[ti][ds(pb, nr), :], p_stg[ds(s0, nr), :]
                        )
                    return nxt

                def av(ci):
                    q0 = NQ * ci
                    prob = prob_bufs[ci % 2]
                    hs_f = st[ci]["hs_f"]
                    for qj in range(NQ // 128):
                        for nh in range(2):
                            o_ps = ops.tile([128, 512], F32, tag="ops", bufs=2)
                            for kt in range(NKT):
                                nc.tensor.matmul(
                                    o_ps,
                                    prob[kt][:, ds(128 * qj, 128)],
                                    m_tiles[kt][:, ds(512 * nh, 512)],
                                    start=(kt == 0),
                                    stop=(kt == NKT - 1),
                                )
                            o_sb = work.tile([128, 512], F32, tag="osb", bufs=3)
                            nc.vector.tensor_add(
                                o_sb, o_ps, hs_f[:, qj, ds(512 * nh, 512)]
                            )
                            nc.sync.dma_start(
                                out[ds(q0 + 128 * qj, 128), ds(512 * nh, 512)],
                                o_sb,
                            )

                load(0)
                for ij in range(C // 128):
                    qt_group(0, ij)
                for ci in range(1, NCHUNK + 1):
                    if ci < NCHUNK:
                        load(ci)
                    for h in range(H):
                        sm_head1(ci - 1, h)
                        if ci < NCHUNK and h % 2 == 0:
                            qt_group(ci, h // 2)
                    d_cur = emit_d(ci - 1, 0)
                    for h in range(H):
                        d_cur = sm_head2(ci - 1, h, d_cur, h + 1 < H)
                    av(ci - 1)

    nc.compile()
    return nc


_NC_CACHE = {}


def get_nc():
    if "nc" not in _NC_CACHE:
        _NC_CACHE["nc"] = build_nc()
    return _NC_CACHE["nc"]


def _bf16(x):
    return np.asarray(x, dtype=ml_dtypes.bfloat16)


def make_in_maps(inputs):
    hs = np.ascontiguousarray(np.asarray(inputs["hidden_states"], dtype=np.float32))
    ehs = np.asarray(inputs["encoder_hidden_states"], dtype=np.float32)
    mask_A = np.asarray(inputs["mask_A"], dtype=np.float32)
    mask_B = np.asarray(inputs["mask_B"], dtype=np.float32)
    Wq = np.asarray(inputs["Wq"], dtype=np.float32)
    Wk = np.asarray(inputs["Wk"], dtype=np.float32)
    Wv = np.asarray(inputs["Wv"], dtype=np.float32)
    Wo = np.asarray(inputs["Wo"], dtype=np.float32)
    bo = np.asarray(inputs["bo"], dtype=np.float32)
    idxA = np.asarray(inputs["token_indices_A"]).astype(np.int64) % T
    idxB = np.asarray(inputs["token_indices_B"]).astype(np.int64) % T

    # suppression as a multiplicative mask: exp(col_bias)^T [77, HW].
    # col_bias "set" semantics: B overwrites A; rows not in A|B are exactly 1.
    col_bias = np.zeros((HW, T), np.float32)
    col_bias[:, idxA] = (-SUPPRESS * (1.0 - mask_A))[:, None]
    col_bias[:, idxB] = (-SUPPRESS * (1.0 - mask_B))[:, None]
    euabt = np.exp(col_bias.T)

    scale = 1.0 / np.sqrt(D)
    wq_bf = _bf16(Wq * scale)
    wk_bf, wv_bf, wo_bf = _bf16(Wk), _bf16(Wv), _bf16(Wo)
    euabt_bf = _bf16(euabt)
    bo_bf = _bf16(bo[None, :])

    in_maps = []
    for b in range(B):
        in_maps.append(
            {
                "hs": hs[b],
                "ehsT": _bf16(ehs[b].T.copy()),
                "wq": wq_bf,
                "wk": wk_bf,
                "wv": wv_bf,
                "wo": wo_bf,
                "euabt": euabt_bf,
                "bo": bo_bf,
            }
        )
    return in_maps


def kernel(**inputs) -> np.ndarray:
    from concourse.bass_utils import run_bass_kernel_spmd

    nc = get_nc()
    in_maps = make_in_maps(inputs)
    res = run_bass_kernel_spmd(nc, in_maps, core_ids=list(range(B)))
    return np.stack([res.results[b]["out"] for b in range(B)]).astype(np.float32)



# revision 6
# speedup vs baseline: 1.4124x; 1.4124x over previous
"""Trainium2 Bass kernel for nn_DenseAttnProcessor (sparse_attention).

Cross-attention block: q = hs@Wq, k/v = ehs@{Wk,Wv}, per-head softmax((q k^T)/8
+ col_bias) @ v, @Wo + bo + residual.  B=8 batches -> data-parallel, one batch
per NeuronCore (no collectives).

v2 dataflow (per core):

  host prep:  hsT fp8 (pre-transposed, so no runtime DMA-transpose), hs
              residual bf16, Wq*8 fp8, Wo*16 bf16, suppression bias factored
              rank-2: ind2 [2,77] (x -SUPPRESS) and mask2 [2,HW] so that
              col_bias^T = ind2^T @ mask2 with exact set-overwrite semantics.
  stage A:    k,v = ehsT^T @ {Wk,Wv} (bf16); kT = PE-transpose(k)/64;
              M_h = v_h @ (16 Wo_h); M rows packed fp8 into [128,10,1024]
              stacked tiles (+ 16*bo row at stacked row 1232).
  stage B (8 chunks of 512 q rows, software-pipelined):
    qT   = (8Wq)^T @ hsT   -- fp8 DoubleRow, 2 k-tiles per matmul
    per head pair: scoresT [77,512] = kT_h^T qT_h (K=64) accumulated with
         the K=2 rank-2 suppression matmul (ind2 x mask2 chunk)
    z    = Exp(scoresT) on Scalar -> DMA-packed into stacked [128,10,512]
    D    = batched over all heads: 10 indicator matmuls -> [16,512] psum
    dinv = 32/D (fast DVE reciprocal, x32 folded at the bf16 copy)
    dexp = indicator-expand matmul back to [128,512] per k-tile
    prob = z * dexp  (DVE, fp8 out, = 32*softmax)
    AV   = prob^T @ M  -- fp8 DoubleRow, 5 matmuls per [128,512] psum
    out  = psum/512 + residual (fused DVE scalar_tensor_tensor), bf16 store.

AV of chunk ci is interleaved into the head loop of chunk ci+1 to keep the PE
stream dense (HAM stays warm).
"""

import sys

for _p in ("/opt/trn_rl_repo",):
    if _p not in sys.path:
        sys.path.insert(0, _p)

import numpy as np
import ml_dtypes

import concourse.mybir as mybir
import concourse.tile as tile
from concourse import bacc
from concourse.bass import ds
from concourse.masks import make_identity

F32 = mybir.dt.float32
BF16 = mybir.dt.bfloat16
FP8 = mybir.dt.float8e4
AF = mybir.ActivationFunctionType
ALU = mybir.AluOpType
DR = mybir.MatmulPerfMode.DoubleRow

B, HW, C, CT, T, H, D = 8, 4096, 1024, 2048, 77, 16, 64
SUPPRESS = 20.0
RT = H * T + 1                # 1233 stacked rows (16*77 head rows + bo row)
NKT = (RT + 127) // 128       # 10 K-tiles for the AV matmul
NQ = 512                      # q rows per chunk
NCHUNK = HW // NQ             # 8
BO_TILE, BO_PART = (H * T) // 128, (H * T) % 128   # bo/ones row: tile 9, p 80
PSC = 32.0                    # probs scale (fp8 range)
MSC = 16.0                    # M scale (fp8 range)
QSC = 64.0                    # q scale (Wq*scale*64 fp8, kT/64)


def _pack_pieces(h):
    """DMA pieces for packing head h's 77 rows at stacked row 77*h, split at
    128-row tile boundaries.  Returns list of (tile_idx, part_base, src_start,
    nrows)."""
    g = T * h
    pieces = []
    pos = 0
    while pos < T:
        gg = g + pos
        ti, d = gg // 128, gg % 128
        n = min(T - pos, 128 - d)
        pieces.append((ti, d, pos, n))
        pos += n
    return pieces


def build_nc():
    nc = bacc.Bacc("TRN2", target_bir_lowering=False, debug=False)

    hsT = nc.dram_tensor("hsT", [C, HW], FP8, kind="ExternalInput")
    hsres = nc.dram_tensor("hsres", [HW, C], BF16, kind="ExternalInput")
    ehsT = nc.dram_tensor("ehsT", [CT, T], BF16, kind="ExternalInput")
    wq = nc.dram_tensor("wq", [C, C], FP8, kind="ExternalInput")
    wk = nc.dram_tensor("wk", [CT, C], BF16, kind="ExternalInput")
    wv = nc.dram_tensor("wv", [CT, C], BF16, kind="ExternalInput")
    wo = nc.dram_tensor("wo", [C, C], BF16, kind="ExternalInput")
    bo = nc.dram_tensor("bo", [1, C], FP8, kind="ExternalInput")
    ind2 = nc.dram_tensor("ind2", [2, T], BF16, kind="ExternalInput")
    mask2 = nc.dram_tensor("mask2", [2, HW], BF16, kind="ExternalInput")
    etd = nc.dram_tensor("etd", [128, NKT * 16], BF16, kind="ExternalInput")
    exp_ind = nc.dram_tensor("exp_ind", [17, NKT * 128], BF16, kind="ExternalInput")
    out = nc.dram_tensor("out", [HW, C], BF16, kind="ExternalOutput")

    with tile.TileContext(nc) as tc:
        with (
            tc.tile_pool(name="const", bufs=1) as const,
            tc.tile_pool(name="persist", bufs=1) as persist,
        ):
            ident = const.tile([128, 128], BF16)
            make_identity(nc, ident)
            ind2_sb = const.tile([2, T], BF16)
            nc.sync.dma_start(ind2_sb, ind2[:, :])
            mask2_sb = const.tile([2, HW], BF16)
            nc.sync.dma_start(mask2_sb, mask2[:, :])
            etd_sb = const.tile([128, NKT, 16], BF16)
            nc.sync.dma_start(etd_sb, etd[:, :])
            ex_sb = const.tile([17, NKT, 128], BF16)
            nc.sync.dma_start(ex_sb, exp_ind[:, :])

            # persistent stacks
            kT_sb = persist.tile([128, C // 128, T], BF16)        # [inner, t]
            m_f8 = persist.tile([128, NKT, C], FP8)               # stacked 16*M
            wq_sb = persist.tile([128, C // 128, C], FP8)
            for i in range(C // 128):
                nc.sync.dma_start(wq_sb[:, i, :], wq[ds(128 * i, 128), :])
            z_bufs = [persist.tile([128, NKT, NQ], BF16, name=f"z{b}") for b in range(2)]
            prob_bufs = [persist.tile([128, NKT, NQ], FP8, name=f"pb{b}") for b in range(2)]
            psc_row = const.tile([1, NQ], BF16)
            nc.any.memset(psc_row, PSC)
            for zb in z_bufs:
                # bo/ones pseudo-row = PSC; rows past it zero (NaN hygiene for
                # the D matmul which reads all 128 partitions).  memset can
                # only start at 32-aligned partitions; DMA patches row 80.
                nc.any.memset(zb[ds(64, 64), BO_TILE, :], 0.0)
                nc.sync.dma_start(zb[ds(BO_PART, 1), BO_TILE, :], psc_row)
            # M stack tile 9: rows past head rows; bo*16 at BO_PART
            nc.any.memset(m_f8[ds(64, 64), BO_TILE, :], 0.0)
            nc.sync.dma_start(m_f8[ds(BO_PART, 1), BO_TILE, :], bo[:, :])

            st = {}

            with (
                tc.tile_pool(name="hsp", bufs=2) as hsp,
                tc.tile_pool(name="work", bufs=2) as work,
                tc.tile_pool(name="soft", bufs=4) as soft,
            ):

                def load(ci):
                    q0 = NQ * ci
                    hsT_t = hsp.tile([128, C // 128, NQ], FP8, tag="hsT")
                    for cj in range(C // 128):
                        nc.sync.dma_start(
                            hsT_t[:, cj, :], hsT[ds(128 * cj, 128), ds(q0, NQ)]
                        )
                    res_t = hsp.tile([128, NQ // 128, C], BF16, tag="res", bufs=3)
                    for qj in range(NQ // 128):
                        nc.sync.dma_start(
                            res_t[:, qj, :], hsres[ds(q0 + 128 * qj, 128), :]
                        )
                    qT = work.tile([128, C // 128, NQ], BF16, tag="qT")
                    st[ci] = dict(hsT=hsT_t, res=res_t, qT=qT)

                def qt_group(ci, ij, ps_pool):
                    hsT_t, qT = st[ci]["hsT"], st[ci]["qT"]
                    q_ps = ps_pool.tile([128, NQ], F32, tag="qps", bufs=1)
                    for c2 in range(C // 256):
                        nc.tensor.matmul(
                            q_ps,
                            wq_sb[:, ds(2 * c2, 2), ds(128 * ij, 128)],
                            hsT_t[:, ds(2 * c2, 2), :],
                            start=(c2 == 0),
                            stop=(c2 == C // 256 - 1),
                            perf_mode=DR,
                        )
                    nc.any.tensor_copy(qT[:, ij, :], q_ps)

                # ---------------- stage A: k, v, kT, M ----------------
                with (
                    tc.tile_pool(name="sa_sb", bufs=3) as sa_sb,
                    tc.tile_pool(name="sa_w", bufs=3) as sa_w,
                    tc.tile_pool(name="sa_ps", bufs=2, space="PSUM") as sa_ps,
                ):
                    # chunk-0 loads + qT(0) early so PE/DMA warm up while the
                    # k/v weight tiles stream in
                    load(0)
                    ehsT_sb = sa_sb.tile([128, CT // 128, T], BF16, bufs=1)
                    for j in range(CT // 128):
                        nc.sync.dma_start(ehsT_sb[:, j, :], ehsT[ds(128 * j, 128), :])
                    for ij in range(C // 128):
                        qt_group(0, ij, sa_ps)

                    kv_sb = {}
                    for name, wten in (("k", wk), ("v", wv)):
                        kv_ps = sa_ps.tile([T, C], F32, tag="kvps", bufs=1)
                        for j in range(CT // 128):
                            wt = sa_w.tile([128, C], BF16, tag="wkv")
                            nc.sync.dma_start(wt, wten[ds(128 * j, 128), :])
                            for nh in range(2):
                                nc.tensor.matmul(
                                    kv_ps[:, ds(512 * nh, 512)],
                                    ehsT_sb[:, j, :],
                                    wt[:, ds(512 * nh, 512)],
                                    start=(j == 0),
                                    stop=(j == CT // 128 - 1),
                                )
                        kvs = sa_sb.tile([T, C], BF16, tag=f"{name}sb", bufs=1)
                        if name == "k":
                            # fold 1/QSC so scoresT = (k/64)^T (64 q/8)
                            nc.scalar.activation(kvs, kv_ps, AF.Copy, scale=1.0 / QSC)
                        else:
                            nc.any.tensor_copy(kvs, kv_ps)
                        kv_sb[name] = kvs

                    # kT / vT via PE transpose of 128-column slices
                    vT_sb = sa_sb.tile([128, C // 128, T], BF16, bufs=1)
                    for src, dst in ((kv_sb["k"], kT_sb), (kv_sb["v"], vT_sb)):
                        for i in range(C // 128):
                            tp = sa_ps.tile([128, T], BF16, tag="tpa")
                            nc.tensor.transpose(tp, src[:, ds(128 * i, 128)], ident[:T, :T])
                            nc.any.tensor_copy(dst[:, i, :], tp)

                    # M_h = v_h @ (16 Wo_h), fp8-packed at stacked row 77h
                    wot = None
                    for h in range(H):
                        i, po = h // 2, (h % 2) * 64
                        if h % 2 == 0:
                            wot = sa_w.tile([128, C], BF16, tag="wot")
                            nc.sync.dma_start(wot, wo[ds(128 * i, 128), :])
                        m_ps = sa_ps.tile([T, C], F32, tag="mps", bufs=1)
                        for nh in range(2):
                            nc.tensor.matmul(
                                m_ps[:, ds(512 * nh, 512)],
                                vT_sb[ds(po, 64), i, :],
                                wot[ds(po, 64), ds(512 * nh, 512)],
                                start=True,
                                stop=True,
                            )
                        m_stg = sa_sb.tile([T, C], FP8, tag="mstg")
                        nc.any.tensor_copy(m_stg, m_ps)
                        for (ti, pb, s0, nr) in _pack_pieces(h):
                            nc.gpsimd.dma_start(
                                m_f8[ds(pb, nr), ti, :], m_stg[ds(s0, nr), :]
                            )

                # ---------------- stage B ----------------
                with tc.tile_pool(name="ops", bufs=2, space="PSUM") as ops:

                    def sm_pair(ci, pair):
                        q0 = NQ * ci
                        qT = st[ci]["qT"]
                        zb = z_bufs[ci % 2]
                        sps = []
                        for sub in range(2):
                            po = 64 * sub
                            sT_ps = ops.tile([T, NQ], F32, tag="sT", bufs=2)
                            nc.tensor.matmul(
                                sT_ps,
                                kT_sb[ds(po, 64), pair, :],
                                qT[ds(po, 64), pair, :],
                                start=True,
                                stop=False,
                            )
                            sps.append(sT_ps)
                        for sT_ps in sps:
                            nc.tensor.matmul(
                                sT_ps,
                                ind2_sb,
                                mask2_sb[:, ds(q0, NQ)],
                                start=False,
                                stop=True,
                            )
                        for sub in range(2):
                            h = 2 * pair + sub
                            z_h = soft.tile([T, NQ], BF16, tag="zh", bufs=4)
                            nc.scalar.activation(z_h, sps[sub], AF.Exp)
                            for (ti, pb, s0, nr) in _pack_pieces(h):
                                nc.sync.dma_start(
                                    zb[ds(pb, nr), ti, :], z_h[ds(s0, nr), :]
                                )

                    def emit_d(ci):
                        zb = z_bufs[ci % 2]
                        d_ps = ops.tile([16, NQ], F32, tag="dps", bufs=1)
                        for kt in range(NKT):
                            nc.tensor.matmul(
                                d_ps,
                                etd_sb[:, kt, :],
                                zb[:, kt, :],
                                start=(kt == 0),
                                stop=(kt == NKT - 1),
                            )
                        dinv = soft.tile([16, NQ], F32, tag="dinv", bufs=2)
                        nc.vector.reciprocal_approx_fast(dinv, d_ps)
                        dinv_bf = soft.tile([17, NQ], BF16, tag="dinvbf", bufs=2)
                        nc.any.memset(dinv_bf, 1.0)
                        nc.scalar.activation(
                            dinv_bf[ds(0, 16), :], dinv, AF.Copy, scale=PSC
                        )
                        return dinv_bf

                    def expand_norm(ci, dinv_bf):
                        zb = z_bufs[ci % 2]
                        pb = prob_bufs[ci % 2]
                        for kt in range(NKT):
                            dexp_ps = ops.tile([128, NQ], F32, tag="dexp", bufs=2)
                            nc.tensor.matmul(
                                dexp_ps, ex_sb[:, kt, :], dinv_bf, start=True, stop=True
                            )
                            nc.vector.tensor_mul(pb[:, kt, :], zb[:, kt, :], dexp_ps)

                    def av_group(ci, g):
                        q0 = NQ * ci
                        qj, nh = g // 2, g % 2
                        pb = prob_bufs[ci % 2]
                        res_t = st[ci]["res"]
                        o_ps = ops.tile([128, 512], F32, tag="ops", bufs=2)
                        for p5 in range(NKT // 2):
                            nc.tensor.matmul(
                                o_ps,
                                pb[:, ds(2 * p5, 2), ds(128 * qj, 128)],
                                m_f8[:, ds(2 * p5, 2), ds(512 * nh, 512)],
                                start=(p5 == 0),
                                stop=(p5 == NKT // 2 - 1),
                                perf_mode=DR,
                            )
                        if nh == 0:
                            st[ci][f"osb{qj}"] = work.tile(
                                [128, C], BF16, tag="osb", bufs=3, name=f"osb{ci}_{qj}"
                            )
                        o_sb = st[ci][f"osb{qj}"]
                        nc.vector.scalar_tensor_tensor(
                            o_sb[:, ds(512 * nh, 512)],
                            o_ps,
                            1.0 / (PSC * MSC),
                            res_t[:, qj, ds(512 * nh, 512)],
                            op0=ALU.mult,
                            op1=ALU.add,
                        )
                        if nh == 1:
                            nc.sync.dma_start(
                                out[ds(q0 + 128 * qj, 128), :], o_sb
                            )

                    for ci in range(NCHUNK):
                        if ci + 1 < NCHUNK:
                            load(ci + 1)
                        for pair in range(H // 2):
                            sm_pair(ci, pair)
                            if pair < 6 and ci + 1 < NCHUNK:
                                qt_group(ci + 1, pair, ops)
                            if ci > 0:
                                av_group(ci - 1, pair)
                        dinv_bf = emit_d(ci)
                        if ci + 1 < NCHUNK:
                            qt_group(ci + 1, 6, ops)
                            qt_group(ci + 1, 7, ops)
                        expand_norm(ci, dinv_bf)
                    for g in range(8):
                        av_group(NCHUNK - 1, g)

    nc.compile()
    return nc


_NC_CACHE = {}


def get_nc():
    if "nc" not in _NC_CACHE:
        _NC_CACHE["nc"] = build_nc()
    return _NC_CACHE["nc"]


def _bf16(x):
    return np.asarray(x, dtype=ml_dtypes.bfloat16)


def _fp8(x):
    return np.clip(np.asarray(x, np.float32), -240.0, 240.0).astype(
        ml_dtypes.float8_e4m3
    )


def make_in_maps(inputs):
    hs = np.asarray(inputs["hidden_states"], dtype=np.float32)
    ehs = np.asarray(inputs["encoder_hidden_states"], dtype=np.float32)
    mask_A = np.asarray(inputs["mask_A"], dtype=np.float32)
    mask_B = np.asarray(inputs["mask_B"], dtype=np.float32)
    Wq = np.asarray(inputs["Wq"], dtype=np.float32)
    Wk = np.asarray(inputs["Wk"], dtype=np.float32)
    Wv = np.asarray(inputs["Wv"], dtype=np.float32)
    Wo = np.asarray(inputs["Wo"], dtype=np.float32)
    bo = np.asarray(inputs["bo"], dtype=np.float32)
    idxA = np.asarray(inputs["token_indices_A"]).astype(np.int64) % T
    idxB = np.asarray(inputs["token_indices_B"]).astype(np.int64) % T

    # rank-2 suppression: bias[t,q] = ind2[:,t] . mask2[:,q], with B-set
    # overwriting A-set (reference applies A then B)
    inA = np.zeros(T, np.float32)
    inA[idxA] = 1.0
    inB = np.zeros(T, np.float32)
    inB[idxB] = 1.0
    ind2_np = np.stack([-SUPPRESS * inA * (1.0 - inB), -SUPPRESS * inB])
    mask2_np = np.stack([1.0 - mask_A, 1.0 - mask_B])

    # D-sum indicator [p, kt*16+h] and expand indicator [h(17), kt*128+p]
    rows = np.arange(NKT * 128)
    head_of = np.where(rows < H * T, rows // T, -1)
    etd_np = np.zeros((128, NKT * 16), np.float32)
    ex_np = np.zeros((17, NKT * 128), np.float32)
    for kt in range(NKT):
        for p in range(128):
            hh = head_of[kt * 128 + p]
            if 0 <= hh < H:
                etd_np[p, kt * 16 + hh] = 1.0
                ex_np[hh, kt * 128 + p] = 1.0
    ex_np[16, BO_TILE * 128 + BO_PART] = 1.0

    wq_f8 = _fp8(Wq * (QSC / np.sqrt(D)))
    wk_bf, wv_bf = _bf16(Wk), _bf16(Wv * 1.0)
    wo_bf = _bf16(Wo * MSC)
    bo_f8 = _fp8(bo * MSC)[None, :]
    ind2_bf, mask2_bf = _bf16(ind2_np), _bf16(mask2_np)
    etd_bf, ex_bf = _bf16(etd_np), _bf16(ex_np)

    in_maps = []
    for b in range(B):
        in_maps.append(
            {
                "hsT": _fp8(hs[b].T),
                "hsres": _bf16(hs[b]),
                "ehsT": _bf16(ehs[b].T.copy()),
                "wq": wq_f8,
                "wk": wk_bf,
                "wv": wv_bf,
                "wo": wo_bf,
                "bo": bo_f8,
                "ind2": ind2_bf,
                "mask2": mask2_bf,
                "etd": etd_bf,
                "exp_ind": ex_bf,
            }
        )
    return in_maps


def kernel(**inputs) -> np.ndarray:
    from concourse.bass_utils import run_bass_kernel_spmd

    nc = get_nc()
    in_maps = make_in_maps(inputs)
    res = run_bass_kernel_spmd(nc, in_maps, core_ids=list(range(B)))
    return np.stack([res.results[b]["out"] for b in range(B)]).astype(np.float32)
